# revision 6
# baseline (speedup 1.0000x reference)
"""Single-head memory attention on Trainium2, batch-parallel across 8 NeuronCores.

Host-side prep (per batch element): cast to bf16 and pack each operand in
the exact SBUF tile layout the PE consumes, so every DMA is a plain
[128 x contiguous] transfer (128 descriptors, ~0.6us trigger) and no
on-chip transposes/casts/rearranges exist at all:
    wq_p[(et p), (dt c)]   = Wq[et*128+c, dt*128+p]     (stationary blocks)
    keys_p[(kt p), (et c)] = keys[kt*128+c, et*128+p]   (stationary blocks)
    x_p[(qc p), (dt q')]   = x[qc*512+q', dt*128+p]     (moving chunks)
    v_p[k, (g c')]         = values[k, g*256+c], c'<256; ones at c'=256
    bq_p[p, t] = bq[t*128+p];  mask_p[p, t] = mask[t*128+p]

Per core (one batch element), bf16 matmuls with fp32 PSUM accumulation:
    QT = WqT.T @ xT + bq                  (MM1: contraction d on partitions)
    ST = keysT.T @ QT                     (MM2: contraction e on partitions)
    ET = exp(ST/sqrt(d) + mask_k)         (one ScalarE activation per kt tile)
    MM3 streams V in 4 groups of 257 cols (256 V cols + a ones col), so each
    PSUM group lands [O_part | sum_k E] and the softmax denominator falls out
    of the same accumulation — no separate ones pass, no sums transpose.
    O = (E.T @ V) * recip(denominator)    (per-partition normalize, bf16 out)

DMA triggers are split across the two HWDGE queues (Sync and Scalar) so the
prologue's trigger chain is not serialized on one engine.
"""

import numpy as np
import ml_dtypes

import concourse.bacc as bacc
import concourse.mybir as mybir
from concourse.tile import TileContext
from concourse.bass_utils import run_bass_kernel_spmd

B, LQ, LK, D = 8, 2048, 2048, 1024
P = 128
QCH = 512                 # queries processed per chunk
NQC = LQ // QCH           # 4 chunks
NDT = D // P              # 8 tiles along d (contraction of MM1)
NET = D // P              # 8 tiles along e (contraction of MM2)
NKT = LK // P             # 16 tiles along k (contraction of MM3)
NQS = QCH // P            # 4 query subtiles per chunk
GW = 256                  # MM3 value-column group width
NG = D // GW              # 4 groups; each streams GW V cols + 1 ones col
SCALE = 1.0 / float(np.sqrt(D))

F32 = mybir.dt.float32
BF16 = mybir.dt.bfloat16
AFT = mybir.ActivationFunctionType

_CACHE = {}


def build_nc():
    nc = bacc.Bacc(None, target_bir_lowering=False)

    x_d = nc.dram_tensor("x_p", [NQC * P, NDT * QCH], BF16, kind="ExternalInput")
    keys_d = nc.dram_tensor("keys_p", [NKT * P, NET * P], BF16, kind="ExternalInput")
    v_d = nc.dram_tensor("v_p", [LK, NG * (GW + 1)], BF16, kind="ExternalInput")
    wq_d = nc.dram_tensor("wq_p", [NET * P, NDT * P], BF16, kind="ExternalInput")
    mask_d = nc.dram_tensor("mask_p", [P, NKT], F32, kind="ExternalInput")
    bq_d = nc.dram_tensor("bq_p", [P, NDT], F32, kind="ExternalInput")
    out_d = nc.dram_tensor("out", [LQ, D], BF16, kind="ExternalOutput")

    with TileContext(nc) as tc:
        with (
            tc.tile_pool(name="persist", bufs=1) as persist,
            tc.tile_pool(name="xTp", bufs=2) as xTp,
            tc.tile_pool(name="QTp", bufs=2) as QTp,
            tc.tile_pool(name="ETp", bufs=2) as ETp,
            tc.tile_pool(name="osb", bufs=3) as osbp,
            tc.tile_pool(name="rcp", bufs=4) as rcp,
            tc.tile_pool(name="psAcc", bufs=5, space="PSUM") as psAccp,
            tc.tile_pool(name="psO", bufs=3, space="PSUM") as psOp,
        ):
            # ---- persistent operands ----
            Wq_sb = persist.tile([P, NET, NDT, P], BF16)
            keys_sb = persist.tile([P, NKT, NET, P], BF16)
            Vaug = persist.tile([P, NKT, NG, GW + 1], BF16)
            bq_sb = persist.tile([P, NDT], F32)
            mask_sb = persist.tile([P, NKT], F32)

            def x_stage(qc, split):
                xT = xTp.tile([P, NDT, QCH], BF16, tag="xT")
                if split:
                    # chunk 0: 2-dt slices so MM1's first group starts early
                    for h in range(NDT // 2):
                        nc.scalar.dma_start(
                            xT[:, 2 * h:2 * h + 2, :],
                            x_d[qc * P:(qc + 1) * P,
                                2 * h * QCH:(2 * h + 2) * QCH],
                        )
                else:
                    nc.sync.dma_start(xT, x_d[qc * P:(qc + 1) * P, :])
                return xT

            def mm1(xT):
                # QT[e, q] = Wq @ x^T + bq
                QT = QTp.tile([P, NET, QCH], BF16, tag="QT")
                for et in range(NET):
                    pq = psAccp.tile([P, QCH], F32, tag="acc")
                    for dt in range(NDT):
                        nc.tensor.matmul(
                            pq,
                            Wq_sb[:, et, dt, :],
                            xT[:, dt, :],
                            start=(dt == 0),
                            stop=(dt == NDT - 1),
                        )
                    nc.vector.tensor_scalar_add(QT[:, et, :], pq, bq_sb[:, et:et + 1])
                return QT

            def mm2(QT):
                # ST[k, q] = keys @ Q^T ; ET = exp(ST/sqrt(d) + mask_k)
                ET = ETp.tile([P, NKT, QCH], BF16, tag="ET")
                for kt in range(NKT):
                    ps = psAccp.tile([P, QCH], F32, tag="acc")
                    for et in range(NET):
                        nc.tensor.matmul(
                            ps,
                            keys_sb[:, kt, et, :],
                            QT[:, et, :],
                            start=(et == 0),
                            stop=(et == NET - 1),
                        )
                    nc.scalar.activation(
                        ET[:, kt, :], ps, AFT.Exp,
                        bias=mask_sb[:, kt:kt + 1], scale=SCALE,
                    )
                return ET

            def mm3(qc, ET):
                # O[q, dv] = sum_k E[k,q] Vaug[k,dv]; col GW of each group is
                # the denominator; normalize with its reciprocal.
                for qs in range(NQS):
                    osb = osbp.tile([P, D], BF16, tag="osb")
                    rc = rcp.tile([P, 1], F32, tag="rc")
                    for g in range(NG):
                        po = psOp.tile([P, GW + 1], F32, tag="po")
                        for kt in range(NKT):
                            nc.tensor.matmul(
                                po,
                                ET[:, kt, qs * P:(qs + 1) * P],
                                Vaug[:, kt, g, :],
                                start=(kt == 0),
                                stop=(kt == NKT - 1),
                            )
                        if g == 0:
                            nc.vector.reciprocal(rc, po[:, GW:GW + 1])
                        oslice = osb[:, g * GW:(g + 1) * GW]
                        if g % 2 == 0:
                            nc.vector.tensor_scalar_mul(oslice, po[:, 0:GW], rc)
                        else:
                            nc.scalar.activation(
                                oslice, po[:, 0:GW], AFT.Copy,
                                bias=0.0, scale=rc,
                            )
                    nc.sync.dma_start(
                        out_d[qc * QCH + qs * P: qc * QCH + (qs + 1) * P, :],
                        osb,
                    )

            # ---- emission: prologue DMAs split across both HWDGE queues ----
            nc.scalar.dma_start(bq_sb, bq_d[:, :])
            xT_next = x_stage(0, split=True)            # scalar queue
            for et in range(NET):                       # sync queue
                nc.sync.dma_start(
                    Wq_sb[:, et, :, :], wq_d[et * P:(et + 1) * P, :]
                )
            for kt in range(NKT):                       # scalar queue
                nc.scalar.dma_start(
                    keys_sb[:, kt, :, :], keys_d[kt * P:(kt + 1) * P, :]
                )
            nc.sync.dma_start(mask_sb, mask_d[:, :])
            for kt in range(NKT):                       # sync queue
                nc.sync.dma_start(Vaug[:, kt, :, :], v_d[kt * P:(kt + 1) * P, :])

            for qc in range(NQC):
                xT = xT_next
                QT = mm1(xT)
                ET = mm2(QT)
                if qc + 1 < NQC:
                    xT_next = x_stage(qc + 1, split=False)
                mm3(qc, ET)

    nc.finalize()
    return nc


def _get_nc():
    if "nc" not in _CACHE:
        _CACHE["nc"] = build_nc()
    return _CACHE["nc"]


def _prep(x, mem_padding_mask, keys, values, Wq, bq):
    bf = ml_dtypes.bfloat16
    cc = np.ascontiguousarray

    # wq_p[(et p), (dt c)] = Wq[et*128+c, dt*128+p]
    wq_p = cc(
        np.asarray(Wq, dtype=np.float32)
        .reshape(NET, P, NDT, P).transpose(0, 3, 2, 1)
        .reshape(NET * P, NDT * P).astype(bf)
    )
    bq_p = cc(np.asarray(bq, dtype=np.float32).reshape(NDT, P).T)
    ones = np.ones((LK, NG, 1), dtype=np.float32)
    maps = []
    for b in range(B):
        x_p = (
            np.asarray(x[b], dtype=np.float32)
            .reshape(NQC, QCH, NDT, P).transpose(0, 3, 2, 1)
            .reshape(NQC * P, NDT * QCH).astype(bf)
        )
        keys_p = (
            np.asarray(keys[b], dtype=np.float32)
            .reshape(NKT, P, NET, P).transpose(0, 3, 2, 1)
            .reshape(NKT * P, NET * P).astype(bf)
        )
        v_p = np.concatenate(
            [np.asarray(values[b], dtype=np.float32).reshape(LK, NG, GW), ones],
            axis=2,
        ).reshape(LK, NG * (GW + 1)).astype(bf)
        mask_p = np.asarray(mem_padding_mask[b], dtype=np.float32).reshape(NKT, P).T
        maps.append({
            "x_p": cc(x_p),
            "keys_p": cc(keys_p),
            "v_p": cc(v_p),
            "mask_p": cc(mask_p),
            "wq_p": wq_p,
            "bq_p": bq_p,
        })
    return maps


def kernel(x, mem_padding_mask, keys, values, Wq, bq):
    nc = _get_nc()
    in_maps = _prep(x, mem_padding_mask, keys, values, Wq, bq)
    res = run_bass_kernel_spmd(nc, in_maps, core_ids=list(range(B)))
    return np.stack(
        [res.results[i]["out"] for i in range(B)], axis=0
    ).astype(np.float32)


# revision 17
# speedup vs baseline: 1.0599x; 1.0599x over previous
"""Single-head memory attention on Trainium2, batch-parallel across 8 NeuronCores.

Host-side prep (per batch element): cast to bf16 and pack each operand in
the exact SBUF tile layout the PE consumes, so every DMA is a plain
[128 x contiguous] transfer (128 descriptors, ~0.6us trigger) and no
on-chip transposes/casts/rearranges exist at all:
    wq_p[(et p), (dt c)]   = Wq[et*128+c, dt*128+p]     (stationary blocks)
    keys_p[(kt p), (et c)] = keys[kt*128+c, et*128+p]   (stationary blocks)
    x_p[(qc p), (dt q')]   = x[qc*512+q', dt*128+p]     (moving chunks)
    v_p[k, (g c')]         = values[k, g*256+c], c'<256; ones at c'=256
    bq_p[p, t] = bq[t*128+p];  mask_p[p, t] = mask[t*128+p]

Per core (one batch element), bf16 matmuls with fp32 PSUM accumulation:
    QT = WqT.T @ xT + bq                  (MM1: contraction d on partitions)
    ST = keysT.T @ QT                     (MM2: contraction e on partitions)
    ET = exp(ST/sqrt(d) + mask_k)         (one ScalarE activation per kt tile)
    MM3 streams V in 4 groups of 257 cols (256 V cols + a ones col), so each
    PSUM group lands [O_part | sum_k E] and the softmax denominator falls out
    of the same accumulation — no separate ones pass, no sums transpose.
    O = (E.T @ V) * recip(denominator)    (per-partition normalize, bf16 out)

Prologue DMAs are emitted on the Sync queue in strict consumption order
(x0 slices + Wq first, then keys per-block, Vaug last) so the 4-deep DMA
ring delivers the MM1/MM2-critical bytes first; bq/mask ride the Scalar
queue.
"""

import numpy as np
import ml_dtypes

import concourse.bacc as bacc
import concourse.mybir as mybir
from concourse.tile import TileContext
from concourse.bass_utils import run_bass_kernel_spmd

B, LQ, LK, D = 8, 2048, 2048, 1024
P = 128
QCH = 512                 # queries processed per chunk
NQC = LQ // QCH           # 4 chunks
NDT = D // P              # 8 tiles along d (contraction of MM1)
NET = D // P              # 8 tiles along e (contraction of MM2)
NKT = LK // P             # 16 tiles along k (contraction of MM3)
NQS = QCH // P            # 4 query subtiles per chunk
GW = 256                  # MM3 value-column group width
NG = D // GW              # 4 groups; each streams GW V cols + 1 ones col
SCALE = 1.0 / float(np.sqrt(D))

F32 = mybir.dt.float32
BF16 = mybir.dt.bfloat16
AFT = mybir.ActivationFunctionType

_CACHE = {}


def build_nc(nkt=NKT):
    nc = bacc.Bacc(None, target_bir_lowering=False)

    x_d = nc.dram_tensor("x_p", [NQC * P, NDT * QCH], BF16, kind="ExternalInput")
    keys_d = nc.dram_tensor("keys_p", [nkt * P, NET * P], BF16, kind="ExternalInput")
    v_d = nc.dram_tensor("v_p", [nkt * P, NG * (GW + 1)], BF16, kind="ExternalInput")
    wq_d = nc.dram_tensor("wq_p", [NET * P, NDT * P], BF16, kind="ExternalInput")
    mask_d = nc.dram_tensor("mask_p", [P, nkt], F32, kind="ExternalInput")
    bq_d = nc.dram_tensor("bq_p", [P, NDT], F32, kind="ExternalInput")
    out_d = nc.dram_tensor("out", [LQ, D], BF16, kind="ExternalOutput")

    with TileContext(nc) as tc:
        with (
            tc.tile_pool(name="persist", bufs=1) as persist,
            tc.tile_pool(name="xTp", bufs=2) as xTp,
            tc.tile_pool(name="QTp", bufs=2) as QTp,
            tc.tile_pool(name="ETp", bufs=2) as ETp,
            tc.tile_pool(name="osb", bufs=3) as osbp,
            tc.tile_pool(name="rcp", bufs=4) as rcp,
            tc.tile_pool(name="psAcc", bufs=5, space="PSUM") as psAccp,
            tc.tile_pool(name="psO", bufs=3, space="PSUM") as psOp,
        ):
            # ---- persistent operands ----
            Wq_sb = persist.tile([P, NET, NDT, P], BF16)
            keys_sb = persist.tile([P, nkt, NET, P], BF16)
            Vaug = persist.tile([P, nkt, NG, GW + 1], BF16)
            bq_sb = persist.tile([P, NDT], F32)
            mask_sb = persist.tile([P, nkt], F32)

            def x_slice(xT, qc, h):
                nc.sync.dma_start(
                    xT[:, 2 * h:2 * h + 2, :],
                    x_d[qc * P:(qc + 1) * P, 2 * h * QCH:(2 * h + 2) * QCH],
                )

            def x_stage(qc):
                xT = xTp.tile([P, NDT, QCH], BF16, tag="xT")
                nc.sync.dma_start(xT, x_d[qc * P:(qc + 1) * P, :])
                return xT

            def mm1(xT):
                # QT[e, q] = Wq @ x^T + bq
                QT = QTp.tile([P, NET, QCH], BF16, tag="QT")
                for et in range(NET):
                    pq = psAccp.tile([P, QCH], F32, tag="acc")
                    for dt in range(NDT):
                        nc.tensor.matmul(
                            pq,
                            Wq_sb[:, et, dt, :],
                            xT[:, dt, :],
                            start=(dt == 0),
                            stop=(dt == NDT - 1),
                        )
                    nc.vector.tensor_scalar_add(QT[:, et, :], pq, bq_sb[:, et:et + 1])
                return QT

            def mm2(QT):
                # ST[k, q] = keys @ Q^T ; ET = exp(ST/sqrt(d) + mask_k)
                ET = ETp.tile([P, nkt, QCH], BF16, tag="ET")
                for kt in range(nkt):
                    ps = psAccp.tile([P, QCH], F32, tag="acc")
                    for et in range(NET):
                        nc.tensor.matmul(
                            ps,
                            keys_sb[:, kt, et, :],
                            QT[:, et, :],
                            start=(et == 0),
                            stop=(et == NET - 1),
                        )
                    nc.scalar.activation(
                        ET[:, kt, :], ps, AFT.Exp,
                        bias=mask_sb[:, kt:kt + 1], scale=SCALE,
                    )
                return ET

            def mm3(qc, ET):
                # O[q, dv] = sum_k E[k,q] Vaug[k,dv]; col GW of each group is
                # the denominator; normalize with its reciprocal.
                for qs in range(NQS):
                    osb = osbp.tile([P, D], BF16, tag="osb")
                    rc = rcp.tile([P, 1], F32, tag="rc")
                    for g in range(NG):
                        po = psOp.tile([P, GW + 1], F32, tag="po")
                        for kt in range(nkt):
                            nc.tensor.matmul(
                                po,
                                ET[:, kt, qs * P:(qs + 1) * P],
                                Vaug[:, kt, g, :],
                                start=(kt == 0),
                                stop=(kt == nkt - 1),
                            )
                        if g == 0:
                            nc.vector.reciprocal(rc, po[:, GW:GW + 1])
                        oslice = osb[:, g * GW:(g + 1) * GW]
                        if g % 2 == 0:
                            nc.vector.tensor_scalar_mul(oslice, po[:, 0:GW], rc)
                        else:
                            nc.scalar.activation(
                                oslice, po[:, 0:GW], AFT.Copy,
                                bias=0.0, scale=rc,
                            )
                        if qc == NQC - 1 and qs == NQS - 1:
                            # tail: ship each group as soon as it normalizes
                            nc.sync.dma_start(
                                out_d[qc * QCH + qs * P: qc * QCH + (qs + 1) * P,
                                      g * GW:(g + 1) * GW],
                                oslice,
                            )
                    if not (qc == NQC - 1 and qs == NQS - 1):
                        nc.sync.dma_start(
                            out_d[qc * QCH + qs * P: qc * QCH + (qs + 1) * P, :],
                            osb,
                        )

            # ---- emission: one queue, strict consumption order. The 4-deep
            # DMA ring throttles in-flight transfers, so arrival order ==
            # trigger order: x0/Wq first (gates MM1), keys stream behind
            # (consumed per-block by MM2), Vaug (needed ~50us in) last ----
            nc.scalar.dma_start(bq_sb, bq_d[:, :])
            xT_next = xTp.tile([P, NDT, QCH], BF16, tag="xT")
            x_slice(xT_next, 0, 0)
            nc.sync.dma_start(Wq_sb[:, 0, :, :], wq_d[0:P, :])
            for h in range(1, NDT // 2):
                x_slice(xT_next, 0, h)
            for et in range(1, NET):
                nc.sync.dma_start(
                    Wq_sb[:, et, :, :], wq_d[et * P:(et + 1) * P, :]
                )
            nc.scalar.dma_start(mask_sb, mask_d[:, :])
            for kt in range(nkt):
                nc.sync.dma_start(
                    keys_sb[:, kt, :, :], keys_d[kt * P:(kt + 1) * P, :]
                )
            for kt in range(nkt):
                nc.sync.dma_start(Vaug[:, kt, :, :], v_d[kt * P:(kt + 1) * P, :])

            for qc in range(NQC):
                xT = xT_next
                QT = mm1(xT)
                ET = mm2(QT)
                if qc + 1 < NQC:
                    xT_next = x_stage(qc + 1)
                mm3(qc, ET)

    nc.finalize()
    return nc


def _get_nc(nkt=NKT):
    if nkt not in _CACHE:
        _CACHE[nkt] = build_nc(nkt)
    return _CACHE[nkt]


def _prep(x, mem_padding_mask, keys, values, Wq, bq):
    """Pack inputs into PE-consumption layouts. Masked keys (exp -> 0) are
    compacted away on the host: softmax and the V-reduction are
    permutation-invariant over k, so dropping -1e9-masked rows and padding
    to a whole number of 128-k tiles is exact. Returns (in_maps, nkt)."""
    bf = ml_dtypes.bfloat16
    cc = np.ascontiguousarray

    masks = [
        np.asarray(mem_padding_mask[b], dtype=np.float32).reshape(LK)
        for b in range(B)
    ]
    valid = [np.where(m > -1e8)[0] for m in masks]
    nkt = max(1, -(-max(len(v) for v in valid) // P))
    nkt = min(nkt, NKT)
    lkc = nkt * P

    # wq_p[(et p), (dt c)] = Wq[et*128+c, dt*128+p]
    wq_p = cc(
        np.asarray(Wq, dtype=np.float32)
        .reshape(NET, P, NDT, P).transpose(0, 3, 2, 1)
        .reshape(NET * P, NDT * P).astype(bf)
    )
    bq_p = cc(np.asarray(bq, dtype=np.float32).reshape(NDT, P).T)
    ones = np.ones((lkc, NG, 1), dtype=np.float32)
    maps = []
    for b in range(B):
        nv = len(valid[b])
        kc = np.zeros((lkc, D), dtype=np.float32)
        kc[:nv] = np.asarray(keys[b], dtype=np.float32)[valid[b]]
        vc = np.zeros((lkc, D), dtype=np.float32)
        vc[:nv] = np.asarray(values[b], dtype=np.float32)[valid[b]]
        mc = np.full(lkc, -1e9, dtype=np.float32)
        mc[:nv] = masks[b][valid[b]]

        x_p = (
            np.asarray(x[b], dtype=np.float32)
            .reshape(NQC, QCH, NDT, P).transpose(0, 3, 2, 1)
            .reshape(NQC * P, NDT * QCH).astype(bf)
        )
        keys_p = (
            kc.reshape(nkt, P, NET, P).transpose(0, 3, 2, 1)
            .reshape(nkt * P, NET * P).astype(bf)
        )
        v_p = np.concatenate(
            [vc.reshape(lkc, NG, GW), ones], axis=2
        ).reshape(lkc, NG * (GW + 1)).astype(bf)
        mask_p = mc.reshape(nkt, P).T
        maps.append({
            "x_p": cc(x_p),
            "keys_p": cc(keys_p),
            "v_p": cc(v_p),
            "mask_p": cc(mask_p),
            "wq_p": wq_p,
            "bq_p": bq_p,
        })
    return maps, nkt


def kernel(x, mem_padding_mask, keys, values, Wq, bq):
    in_maps, nkt = _prep(x, mem_padding_mask, keys, values, Wq, bq)
    nc = _get_nc(nkt)
    res = run_bass_kernel_spmd(nc, in_maps, core_ids=list(range(B)))
    return np.stack(
        [res.results[i]["out"] for i in range(B)], axis=0
    ).astype(np.float32)
